# revision 1
# baseline (speedup 1.0000x reference)
"""Trainium2 Bass kernel v3 for the Sobel magnitude-gradient-error loss
(nn_MGE): mean(|sqrt-diff of Sobel magnitudes|) over [64,1,512,512] pairs.

Distribution: pure data-parallel, batch 64 split as 8 pairs (16 images) per
NeuronCore; each core emits per-partition partial sums [128, 9]; host reduces.

v3 structure:
  - x loaded NON-overlapped [128, 4, 512] in one fat DMA per image (the
    collapsible access pattern runs ~3x faster than any overlapped-row load).
  - p[c] = x[c-1]+x[c] (DVE, 513 wide via 2 tiny edge copies); A = p0+p1,
    B = p1-p0 materialized contiguous -> aligned matmul inputs.
  - gh = D@A, gv = S@B per 128-row block: 8 matmuls/image, no halo matmuls.
    D/S columns 0,127 are ZERO: block-boundary out-rows (8 per image) produce
    0 and are instead computed by a packed FIXUP pass: 16 in-rows per image
    x 8 images = one [128, 512] tile per tensor, 2 matmuls, own drains.
  - m2 = gv^2 + sqh via DVE SQADD1 (z-fraction via ACT sqv + DVE add);
    sqh = gh^2 on ACT; mag = sqrt(m2) on ACT; |mag_t - mag_p| + accumulate
    via DVE ABSDIFF custom op.
  - Software-pipelined emission: loads+p+A/B lead, matmuls lag one image,
    PSUM drains one more (in-order engine queues).
"""

import sys
import types

sys.path.insert(0, "/opt/trn_rl_repo")

import numpy as np

# ---------------------------------------------------------------- axon NTFF
if "antenv.axon_hooks" not in sys.modules:
    _m = types.ModuleType("antenv.axon_hooks")
    _m._h = None
    _m.set_axon_ntff_profile_hook = lambda h: setattr(_m, "_h", h)
    _m.get_axon_ntff_profile_hook = lambda: _m._h
    sys.modules["antenv.axon_hooks"] = _m
    try:
        import antenv

        antenv.axon_hooks = _m
    except Exception:
        pass

import ml_dtypes
import concourse.bass as bass
import concourse.tile as tile
from concourse.ap import AP
from concourse import bacc, mybir
import concourse.bass_utils as bass_utils
import concourse.dve_ops as dve_ops
from concourse.dve_ops import DveOp, OPS
from concourse.dve_spec import (
    Spec,
    Src0,
    Src1,
    C0,
    Zero,
    sq,
    maxx,
    lower,
    AluOp,
    _has_src1,
)
from concourse.dve_uop import DveOpSpec

bass_utils.upload_artifacts = lambda tmpdir: "local://skipped"

N_CORES = 8
PAIRS_PER_CORE = 8
N_IMG = 2 * PAIRS_PER_CORE  # 16 images per core
H = W = 512
NBLK = 4  # 128-row blocks
FP32 = mybir.dt.float32
BF16 = mybir.dt.bfloat16

# dead out-rows per image (zeroed by D/S cols 0,127; computed by the fixup):
#   0, 127, 128, 255, 256, 383, 384, 511
# fixup in-rows per image (16): 0,1 | 126..129 | 254..257 | 382..385 | 510,511
FIX_IN_PER_IMG = 16
FIX_OUT_ROWS = [0, 127, 128, 255, 256, 383, 384, 511]
# local k' of each in-row within the 16-row group:
#   0,1 -> 0,1 ; 126..129 -> 2..5 ; 254..257 -> 6..9 ; 382..385 -> 10..13 ;
#   510,511 -> 14,15


def _register_op(name, spec, subdim=False):
    for op in OPS:
        if op.name == name:
            return op
    shas = {}
    rd1 = _has_src1(spec)
    for ver in ("v3", "v4"):
        tmp = DveOpSpec(name=name, opcode=0, uops=lower(spec, ver=ver), rd1_en=rd1)
        shas[ver] = tmp.sha(ver)
    op = DveOp(name, spec, subdim, uops_sha=shas)
    OPS.append(op)
    dve_ops.CUSTOM_DVE_SPECS[name] = spec
    dve_ops._SUB_OPCODE_FOR_NAME[name] = dve_ops._CUSTOM_DVE_ROW_BASE + len(OPS) - 1
    return op


# out = in0^2 + in1  (in0 = gv conv from PSUM, in1 = sqh bf16 in SBUF)
SQADD1 = _register_op(
    "SQADD1_ANT",
    Spec(
        body=sq(Src0) + Src1,
        reference=lambda in0, in1, s0, s1, imm2: in0 * in0 + in1,
    ),
)


# out = |in0 - in1| ; accum_out = s0 + sum(out)
def _absdiff_ref(in0, in1, s0, s1, imm2):
    b = np.abs(in0.astype(np.float32) - in1.astype(np.float32))
    return b, s0 + b.reshape(b.shape[0], -1).sum(axis=-1, keepdims=True)


_d = Src0 - Src1
ABSDIFF = _register_op(
    "ABSDIFF_ACC_ANT",
    Spec(
        body=maxx(_d, Zero - _d),
        accum=AluOp.ADD,
        accum_init=C0,
        reference=_absdiff_ref,
    ),
)


def _band_matrices():
    """[4,128,128] bf16 stationary matrices (lhsT: out[m] = sum_k W[k,m] rhs[k]).

    Main (non-overlapped 128-row blocks, in-row k = row-128c, out row 128c+m):
      gh[r] = A[r+1]-A[r-1] -> D[m+1,m]=+1, D[m-1,m]=-1, cols 0,127 ZERO
      gv[r] = B[r-1]+2B[r]+B[r+1] -> S[m-1,m]=1, S[m,m]=2, S[m+1,m]=1,
      cols 0,127 ZERO (those rows come from the fixup pass).
    Fixup (8 groups of 16 in-rows; out rows FIX_OUT_ROWS at out partition
    16g+j): coefficients per the Sobel SAME-padding boundary rules.
    """
    Dm = np.zeros((128, 128), np.float32)
    Sm = np.zeros((128, 128), np.float32)
    for m in range(1, 127):
        Dm[m - 1, m] = -1.0
        Dm[m + 1, m] = 1.0
        Sm[m - 1, m] = 1.0
        Sm[m, m] = 2.0
        Sm[m + 1, m] = 1.0
    Df = np.zeros((128, 128), np.float32)
    Sf = np.zeros((128, 128), np.float32)
    # k' layout per 16-row group (chosen so each image needs only TWO
    # contiguous-destination DMAs):
    #   k' 0,1  = rows 0,1      k' 2,3   = rows 510,511
    #   k' 4..7 = rows 126..129 k' 8..11 = rows 254..257
    #   k' 12..15 = rows 382..385
    # per group: (out_j, [(k', coef_D)], [(k', coef_S)])
    fix = [
        # out row 0: gh = +A[1]; gv = 2B[0] + B[1]
        (0, [(1, 1.0)], [(0, 2.0), (1, 1.0)]),
        # out 127: gh = A[128]-A[126]; gv = B[126]+2B[127]+B[128]
        (1, [(6, 1.0), (4, -1.0)], [(4, 1.0), (5, 2.0), (6, 1.0)]),
        # out 128: gh = A[129]-A[127]; gv = B[127]+2B[128]+B[129]
        (2, [(7, 1.0), (5, -1.0)], [(5, 1.0), (6, 2.0), (7, 1.0)]),
        # out 255/256: in 254..257 at k' 8..11
        (3, [(10, 1.0), (8, -1.0)], [(8, 1.0), (9, 2.0), (10, 1.0)]),
        (4, [(11, 1.0), (9, -1.0)], [(9, 1.0), (10, 2.0), (11, 1.0)]),
        # out 383/384: in 382..385 at k' 12..15
        (5, [(14, 1.0), (12, -1.0)], [(12, 1.0), (13, 2.0), (14, 1.0)]),
        (6, [(15, 1.0), (13, -1.0)], [(13, 1.0), (14, 2.0), (15, 1.0)]),
        # out 511: gh = -A[510] = -k'2; gv = B[510]+2B[511] = k'2+2k'3
        (7, [(2, -1.0)], [(2, 1.0), (3, 2.0)]),
    ]
    for g in range(PAIRS_PER_CORE):
        o = FIX_IN_PER_IMG * g
        for j, dk, sk in fix:
            for k, c in dk:
                Df[o + k, o + j] = c
            for k, c in sk:
                Sf[o + k, o + j] = c
    return np.stack([Dm, Sm, -Sm, Df, Sf]).astype(ml_dtypes.bfloat16)


_BAND_IDX = {"D": 0, "S": 1, "Sn": 2, "Df": 3, "Sf": 4}

# per-image knobs:
#   ZSEQ[i]=1: m2 via ACT sqv + DVE 2x add (Z-path); 0: DVE SQADD1
#   BPOOL[i]=1: B computed on Pool; 0: on DVE
import os


def _seq(env, default):
    v = os.environ.get(env)
    if not v:
        return default
    if len(v) == 1:
        return [int(v)] * 16
    return [int(c) for c in v]


ZSEQ = _seq("K3_Z", [0] * 16)
BPOOL = _seq("K3_BPOOL", [0] * 16)
# AFOLD[i]=1: A folded into PE (gh = D@p0 + D@p1); 0: A materialized on DVE
AFOLD = _seq("K3_AFOLD", [1] * 16)
N_FILLER_LDW = int(os.environ.get("K3_FILLER", "0"))
ABS_ACT_SEQ = _seq("K3_ABSACT", [0] * 16)  # per-pair (idx j//2): d on DVE + Abs/acc on ACT
# BFOLD[i]=1: gv folded into PE (S@p1 + Sn@p0); 0: B materialized
BFOLD = _seq("K3_BFOLD", [1] * 16)


def build(n_pairs=PAIRS_PER_CORE):
    nc = bacc.Bacc(None, target_bir_lowering=False, debug=False, num_swdge_queues=4)

    yp = nc.dram_tensor("y_p", [n_pairs, H, W], BF16, kind="ExternalInput")
    yt = nc.dram_tensor("y_t", [n_pairs, H, W], BF16, kind="ExternalInput")
    bands = nc.dram_tensor("bands", [128, 5 * 128], BF16, kind="ExternalInput")
    out = nc.dram_tensor("out", [128, n_pairs + 1], FP32, kind="ExternalOutput")

    with tile.TileContext(nc) as tc:
        with (
            tc.tile_pool(name="cst", bufs=1) as cst,
            tc.tile_pool(name="xp", bufs=5) as xp,
            tc.tile_pool(name="pp", bufs=4) as pp,
            tc.tile_pool(name="abp", bufs=3) as abp,
            tc.tile_pool(name="sqp", bufs=4) as sqp,
            tc.tile_pool(name="m2p", bufs=3) as m2p,
            tc.tile_pool(name="magp", bufs=4) as magp,
            tc.tile_pool(name="absp", bufs=2) as absp,
            tc.tile_pool(name="fixp", bufs=1) as fixp,
            tc.tile_pool(name="accp", bufs=1) as accp,
            tc.tile_pool(name="ghp", bufs=2, space="PSUM") as ghp,
            tc.tile_pool(name="gvp", bufs=2, space="PSUM") as gvp,
        ):
            cmats = cst.tile([128, 5, 128], BF16, name="cmats")
            nc.sync.dma_start(cmats[:], bands.rearrange("p (k m) -> p k m", k=5))
            cmat = {n: cmats[:, i, :] for n, i in _BAND_IDX.items()}

            acc = accp.tile([128, n_pairs + 1], FP32, name="acc")
            nc.gpsimd.memset(acc[:], 0.0)


            def make_inputs(i, src, b):
                """x fat-load; p [128,NBLK,513]; A,B [128,NBLK,512]."""
                x = xp.tile([128, NBLK, W], BF16, tag="x", name="x")
                nc.sync.dma_start(x[:], src[b].rearrange("(c p) w -> p c w", p=128))
                p = pp.tile([128, NBLK, W + 1], BF16, tag="p", name="p")
                # p[c'] = x[c'-1] + x[c'] ; edges: p[0]=x[0], p[512]=x[511]
                nc.vector.tensor_add(
                    p[:, :, 1:W], x[:, :, 0 : W - 1], x[:, :, 1:W]
                )
                nc.gpsimd.tensor_scalar_add(p[:, :, 0:1], x[:, :, 0:1], 0.0)
                nc.gpsimd.tensor_scalar_add(
                    p[:, :, W : W + 1], x[:, :, W - 1 : W], 0.0
                )
                A = None
                if not AFOLD[i]:
                    A = abp.tile([128, NBLK, W], BF16, tag="A", name="A")
                    nc.vector.tensor_add(A[:], p[:, :, 0:W], p[:, :, 1 : W + 1])
                B = None
                if not BFOLD[i]:
                    B = abp.tile([128, NBLK, W], BF16, tag="B", name="B")
                    eng = nc.gpsimd if BPOOL[i] else nc.vector
                    eng.tensor_tensor(
                        B[:], p[:, :, 1 : W + 1], p[:, :, 0:W],
                        mybir.AluOpType.subtract,
                    )
                return p, A, B

            def conv_image(i, pAB):
                """8-12 matmuls; returns ([gh h0,h1],[gv h0,h1]) [128,2W] PSUM."""
                p, A, B = pAB
                for _ in range(N_FILLER_LDW):
                    nc.tensor.ldweights(cmat["D"])
                ghs = [
                    ghp.tile([128, 2 * W], FP32, tag="gh", name="gh") for _ in range(2)
                ]
                gvs = [
                    gvp.tile([128, 2 * W], FP32, tag="gv", name="gv") for _ in range(2)
                ]

                def seg(tiles, blk):
                    h, u = blk // 2, blk % 2
                    return tiles[h][:, u * W : (u + 1) * W]

                if A is None:
                    for blk in range(NBLK):
                        nc.tensor.matmul(
                            seg(ghs, blk), cmat["D"], p[:, blk, 0:W],
                            start=True, stop=False,
                        )
                        nc.tensor.matmul(
                            seg(ghs, blk), cmat["D"], p[:, blk, 1 : W + 1],
                            start=False, stop=True,
                        )
                else:
                    for blk in range(NBLK):
                        nc.tensor.matmul(
                            seg(ghs, blk), cmat["D"], A[:, blk, :],
                            start=True, stop=True,
                        )
                if B is None:
                    for blk in range(NBLK):
                        nc.tensor.matmul(
                            seg(gvs, blk), cmat["S"], p[:, blk, 1 : W + 1],
                            start=True, stop=False,
                        )
                    for blk in range(NBLK):
                        nc.tensor.matmul(
                            seg(gvs, blk), cmat["Sn"], p[:, blk, 0:W],
                            start=False, stop=True,
                        )
                else:
                    for blk in range(NBLK):
                        nc.tensor.matmul(
                            seg(gvs, blk), cmat["S"], B[:, blk, :],
                            start=True, stop=True,
                        )
                return ghs, gvs

            def magnitude(i, ghs, gvs):
                """sqh -> m2 -> mag for one image; returns mag [128,NBLK,W]."""
                m2 = m2p.tile([128, NBLK, W], BF16, tag="m2", name="m2")
                mag = magp.tile([128, NBLK, W], BF16, tag="mag", name="mag")
                m2v = m2.rearrange("q (h u) w -> q h (u w)", u=2)
                for h in range(2):
                    sqh = sqp.tile([128, 2 * W], BF16, tag="sqh", name="sqh")
                    nc.scalar.square(sqh[:], ghs[h][:])
                    if ZSEQ[i]:
                        sqv = sqp.tile([128, 2 * W], BF16, tag="sqv", name="sqv")
                        nc.scalar.square(sqv[:], gvs[h][:])
                        nc.vector.tensor_add(m2v[:, h, :], sqh[:], sqv[:])
                    else:
                        nc.vector._custom_dve(
                            SQADD1, out=m2v[:, h, :], in0=gvs[h][:], in1=sqh[:]
                        )
                nc.scalar.activation(
                    mag.rearrange("q b w -> q (b w)"),
                    m2.rearrange("q b w -> q (b w)"),
                    mybir.ActivationFunctionType.Sqrt,
                )
                return mag

            # ---------------- fixup pass: boundary rows of all 8 images
            fmags = []
            fxt = {}

            def fixup_alloc(ti):
                fxt[ti] = fixp.tile([128, W], BF16, tag=f"xf{ti}", name="xf")

            def fixup_dma(ti, src, g):
                """Two DMAs for image-group g into the fixup tile."""
                xf = fxt[ti]
                img = src[g]
                o = FIX_IN_PER_IMG * g
                # k' 0..3 <- rows 0,1,510,511 (two 2-row strips, strip-major)
                sb = AP(img.tensor, img.offset, [[(H - 2) * W, 2], [W, 2], [1, W]])
                nc.gpsimd.dma_start(xf[o : o + 4, :], sb)
                # k' 4..15 <- rows 126..129, 254..257, 382..385 (stride-128)
                s3 = AP(
                    img.tensor,
                    img.offset + 126 * W,
                    [[128 * W, 3], [W, 4], [1, W]],
                )
                nc.gpsimd.dma_start(xf[o + 4 : o + 16, :], s3)

            def fixup_compute(ti):
                xf = fxt[ti]
                pf = fixp.tile([128, W + 1], BF16, tag=f"pf{ti}", name="pf")
                nc.vector.tensor_add(pf[:, 1:W], xf[:, 0 : W - 1], xf[:, 1:W])
                nc.vector.tensor_scalar_add(pf[:, 0:1], xf[:, 0:1], 0.0)
                nc.vector.tensor_scalar_add(pf[:, W : W + 1], xf[:, W - 1 : W], 0.0)
                Af = fixp.tile([128, W], BF16, tag=f"Af{ti}", name="Af")
                nc.vector.tensor_add(Af[:], pf[:, 0:W], pf[:, 1 : W + 1])
                Bf = fixp.tile([128, W], BF16, tag=f"Bf{ti}", name="Bf")
                nc.vector.tensor_tensor(
                    Bf[:], pf[:, 1 : W + 1], pf[:, 0:W], mybir.AluOpType.subtract
                )
                ghf = ghp.tile([128, 2 * W], FP32, tag="gh", name="ghf")
                gvf = gvp.tile([128, 2 * W], FP32, tag="gv", name="gvf")
                nc.tensor.matmul(ghf[:, 0:W], cmat["Df"], Af[:], start=True, stop=True)
                nc.tensor.matmul(gvf[:, 0:W], cmat["Sf"], Bf[:], start=True, stop=True)
                sqhf = fixp.tile([128, W], BF16, tag=f"sqhf{ti}", name="sqhf")
                nc.scalar.square(sqhf[:], ghf[:, 0:W])
                m2f = fixp.tile([128, W], BF16, tag=f"m2f{ti}", name="m2f")
                nc.vector._custom_dve(SQADD1, out=m2f[:], in0=gvf[:, 0:W], in1=sqhf[:])
                magf = fixp.tile([128, W], BF16, tag=f"magf{ti}", name="magf")
                nc.scalar.activation(magf[:], m2f[:], mybir.ActivationFunctionType.Sqrt)
                fmags.append(magf)

            fixup_alloc(0)
            fixup_alloc(1)

            # ---------------- main images, software-pipelined
            ps = {}
            conv = {}
            mags = {}

            def emit_drains(j):
                mags[j] = magnitude(j, *conv.pop(j))

            def emit_absdiff(j):
                scr = absp.tile([128, NBLK, W], BF16, tag="scr", name="scr")
                nc.vector._custom_dve(
                    ABSDIFF,
                    out=scr[:],
                    in0=mags.pop(j)[:],
                    in1=mags.pop(j - 1)[:],
                    s0=0.0,
                    accum_out=acc[:, j // 2 : j // 2 + 1],
                )

            for i in range(N_IMG + 3):
                if i < N_IMG:
                    src = yp if i % 2 == 0 else yt
                    ps[i] = make_inputs(i, src, i // 2)
                # spread the 32 small fixup DMAs through the image stream,
                # AFTER the first 4 image loads have priority in the queue
                if 4 <= i < 8:
                    for g in range(2 * (i - 4), 2 * (i - 3)):
                        fixup_dma(0, yp, g)
                        fixup_dma(1, yt, g)
                if i >= 2 and i - 2 < N_IMG:
                    emit_drains(i - 2)
                # absdiff deferred one step so it never blocks p(i+1) in the
                # in-order DVE queue while waiting on ACT's sqrt
                if i >= 3 and (i - 3) % 2 == 1:
                    emit_absdiff(i - 3)
                # fixup compute sits after the drains so its PSUM allocation
                # follows the frees already in the queues
                if i == 9:
                    fixup_compute(0)
                if i == 11:
                    fixup_compute(1)
                if i == 12:
                    scrf = fixp.tile([128, W], BF16, tag="scrf", name="scrf")
                    nc.vector._custom_dve(
                        ABSDIFF,
                        out=scrf[:],
                        in0=fmags[1][:],
                        in1=fmags[0][:],
                        s0=0.0,
                        accum_out=acc[:, n_pairs : n_pairs + 1],
                    )
                if i >= 1 and i - 1 < N_IMG:
                    j = i - 1
                    conv[j] = conv_image(j, ps.pop(j))

            nc.sync.dma_start(out[:], acc[:])

    nc.compile()
    return nc


_CACHED = {}


def _get_nc(n_pairs=PAIRS_PER_CORE):
    if n_pairs not in _CACHED:
        _CACHED[n_pairs] = build(n_pairs)
    return _CACHED[n_pairs]


def _to_bf16(a):
    return np.ascontiguousarray(a.astype(ml_dtypes.bfloat16))


def kernel(y_p: np.ndarray, y_t: np.ndarray) -> np.ndarray:
    assert y_p.shape == (64, 1, H, W) and y_t.shape == (64, 1, H, W)
    ypf = _to_bf16(np.asarray(y_p).reshape(64, H, W))
    ytf = _to_bf16(np.asarray(y_t).reshape(64, H, W))
    bands = np.ascontiguousarray(
        _band_matrices().transpose(1, 0, 2).reshape(128, 5 * 128)
    )

    nc = _get_nc()
    in_maps = []
    for c in range(N_CORES):
        s = slice(c * PAIRS_PER_CORE, (c + 1) * PAIRS_PER_CORE)
        in_maps.append({"y_p": ypf[s], "y_t": ytf[s], "bands": bands})

    res = bass_utils.run_bass_kernel_spmd(nc, in_maps, core_ids=list(range(N_CORES)))
    total = np.float64(0.0)
    for r in res.results:
        total += np.sum(r["out"].astype(np.float64))
    mean = total / float(64 * H * W)
    return np.float32(mean)

